# revision 35
# baseline (speedup 1.0000x reference)
"""Trainium2 Bass kernel for DecomposableAttention (B=512, L=256, V=50000, E=300, H=200).

Strategy: data-parallel over batch across 8 cores (64 batches/core).  Per batch:
indirect-DMA gather of bf16 embedding rows, on-chip PE transposes to get the
E-on-partitions layout, bf16 matmuls (1 cycle/row) for the attend/compare MLPs
and the attention einsums with fp32 PSUM accumulation, ACT-exp softmaxes in
fp32 with the length masks folded in as per-partition -30000 biases, and a
final aggregate MLP over all 64 batches.  All matmul free dims are 256.

The softmax max-subtraction uses one shared rowmax M broadcast to both
directions; since the same M value multiplies numerator and denominator of
each normalized weight, its precision cancels exactly - so the broadcast can
be bf16.  Reductions/exp/normalization stay fp32.
"""
import os as _os
import sys

if '/opt/trn_rl_repo' not in sys.path:
    sys.path.insert(0, '/opt/trn_rl_repo')

import numpy as np

B, L, VOCAB, EMBED, HIDDEN = 512, 256, 50000, 300, 200
NCORES = 8
BC = B // NCORES  # batches per core

_prog_cache = {}


def build_program(nb=BC):
    import concourse.bass as bass
    import concourse.bacc as bacc
    import concourse.tile as tile
    import concourse.mybir as mybir
    from concourse.masks import make_identity

    F32 = mybir.dt.float32
    BF16 = mybir.dt.bfloat16
    I32 = mybir.dt.int32
    AX = mybir.AxisListType
    ALU = mybir.AluOpType
    ACTF = mybir.ActivationFunctionType
    P = 128
    EK = [(0, 100), (100, 200), (200, 300)]      # E contraction chunks of 100
    H2 = [(0, 100), (100, 200)]                  # H chunks of 100
    # E output chunks for the attention sums / xT / W1cb: 100+104+96 so the
    # ones-augmented last chunk puts the softmax denominator at PSUM partition
    # 96 (ACT reads need 32-aligned base partitions).
    E3 = [(0, 100), (100, 204), (204, 300)]

    nc = bacc.Bacc("TRN2", num_devices=NCORES)

    emb_d = nc.dram_tensor("emb", [VOCAB, EMBED], BF16, kind="ExternalInput")
    s1_d = nc.dram_tensor("s1", [nb, L], I32, kind="ExternalInput")
    s2_d = nc.dram_tensor("s2", [nb, L], I32, kind="ExternalInput")
    len1_d = nc.dram_tensor("len1", [nb, 1], I32, kind="ExternalInput")
    len2_d = nc.dram_tensor("len2", [nb, 1], I32, kind="ExternalInput")
    W1a_d = nc.dram_tensor("W1a", [EMBED, HIDDEN], BF16, kind="ExternalInput")
    W2a_d = nc.dram_tensor("W2a", [HIDDEN, HIDDEN], BF16, kind="ExternalInput")
    W1c_d = nc.dram_tensor("W1c", [2 * EMBED, HIDDEN], BF16, kind="ExternalInput")
    W2c_d = nc.dram_tensor("W2c", [HIDDEN, HIDDEN], BF16, kind="ExternalInput")
    W1g_d = nc.dram_tensor("W1g", [2 * HIDDEN, HIDDEN], BF16, kind="ExternalInput")
    W2g_d = nc.dram_tensor("W2g", [HIDDEN, 2], BF16, kind="ExternalInput")
    b1a_d = nc.dram_tensor("b1a", [HIDDEN, 1], F32, kind="ExternalInput")
    b2a_d = nc.dram_tensor("b2a", [HIDDEN, 1], F32, kind="ExternalInput")
    b1c_d = nc.dram_tensor("b1c", [HIDDEN, 1], F32, kind="ExternalInput")
    b2c_d = nc.dram_tensor("b2c", [HIDDEN, 1], F32, kind="ExternalInput")
    b1g_d = nc.dram_tensor("b1g", [HIDDEN, 1], F32, kind="ExternalInput")
    b2g_d = nc.dram_tensor("b2g", [2, 1], F32, kind="ExternalInput")
    out_d = nc.dram_tensor("out", [nb, 2], F32, kind="ExternalOutput")
    DBG = _os.environ.get("KDBG", "") == "1"
    if DBG:
        vdbg_d = nc.dram_tensor("vdbg", [400, nb], F32, kind="ExternalOutput")

    with tile.TileContext(nc) as tc:
        import contextlib
        ctx = contextlib.ExitStack()
        with ctx:
            const = ctx.enter_context(tc.tile_pool(name="const", bufs=1))
            psA = ctx.enter_context(tc.tile_pool(name="psA", bufs=3, space="PSUM"))
            psH = ctx.enter_context(tc.tile_pool(name="psH", bufs=2, space="PSUM"))
            psS = ctx.enter_context(tc.tile_pool(name="psS", bufs=3, space="PSUM"))
            gat = ctx.enter_context(tc.tile_pool(name="gat", bufs=4))
            eTp = ctx.enter_context(tc.tile_pool(name="eTp", bufs=3))
            hp = ctx.enter_context(tc.tile_pool(name="hp", bufs=3))
            sm = ctx.enter_context(tc.tile_pool(name="sm", bufs=3))
            att = ctx.enter_context(tc.tile_pool(name="att", bufs=3))
            cmp_ = ctx.enter_context(tc.tile_pool(name="cmp", bufs=3))

            # ---------------- constants ----------------
            ident = const.tile([P, P], F32)
            make_identity(nc, ident[:])
            ident_b = const.tile([P, P], BF16)
            nc.vector.tensor_copy(ident_b[:], ident[:])

            ones_f = const.tile([P, 1], F32)
            nc.vector.memset(ones_f[:], 1.0)
            ones_col_b = const.tile([P, 1], BF16)   # lhsT for den sums (K=128, M=1)
            nc.vector.tensor_copy(ones_col_b[:], ones_f[:])
            ones_row_f = const.tile([1, P], F32)
            nc.vector.memset(ones_row_f[:], 1.0)
            ones_row_b = const.tile([1, P], BF16)   # lhsT for bcasts (K=1, M=128)
            nc.vector.tensor_copy(ones_row_b[:], ones_row_f[:])

            # weights (already bf16 in DRAM, cast host-side)
            W1a_t = [const.tile([k1 - k0, HIDDEN], BF16, name=f"W1a{i}", tag=f"W1a{i}") for i, (k0, k1) in enumerate(EK)]
            for i, (k0, k1) in enumerate(EK):
                nc.sync.dma_start(W1a_t[i][:], W1a_d[k0:k1, :])
            W2a_t = [const.tile([100, HIDDEN], BF16, name=f"W2a{i}", tag=f"W2a{i}") for i in range(2)]
            for i, (k0, k1) in enumerate(H2):
                nc.sync.dma_start(W2a_t[i][:], W2a_d[k0:k1, :])
            W1ca_t = [const.tile([k1 - k0, HIDDEN], BF16, name=f"W1ca{i}", tag=f"W1ca{i}") for i, (k0, k1) in enumerate(EK)]
            for i, (k0, k1) in enumerate(EK):
                nc.sync.dma_start(W1ca_t[i][:], W1c_d[k0:k1, :])
            W1cb_t = [const.tile([k1 - k0, HIDDEN], BF16, name=f"W1cb{i}", tag=f"W1cb{i}") for i, (k0, k1) in enumerate(E3)]
            for i, (k0, k1) in enumerate(E3):
                nc.sync.dma_start(W1cb_t[i][:], W1c_d[EMBED + k0:EMBED + k1, :])
            W2c_t = [const.tile([100, HIDDEN], BF16, name=f"W2c{i}", tag=f"W2c{i}") for i in range(2)]
            for i, (k0, k1) in enumerate(H2):
                nc.sync.dma_start(W2c_t[i][:], W2c_d[k0:k1, :])
            W1g_t = [const.tile([100, HIDDEN], BF16, name=f"W1g{i}", tag=f"W1g{i}") for i in range(4)]
            for i in range(4):
                nc.sync.dma_start(W1g_t[i][:], W1g_d[i * 100:(i + 1) * 100, :])
            W2g_t = [const.tile([100, 2], BF16, name=f"W2g{i}", tag=f"W2g{i}") for i in range(2)]
            for i, (k0, k1) in enumerate(H2):
                nc.sync.dma_start(W2g_t[i][:], W2g_d[k0:k1, :])

            def bias2(d):
                t = [const.tile([100, 1], F32, name=f"b{d.name}{i}", tag=f"b{d.name}{i}") for i in range(2)]
                for i, (k0, k1) in enumerate(H2):
                    nc.sync.dma_start(t[i][:], d[k0:k1, :])
                return t
            b1a_t, b2a_t = bias2(b1a_d), bias2(b2a_d)
            b1c_t, b2c_t = bias2(b1c_d), bias2(b2c_d)
            b1g_t = bias2(b1g_d)
            b2g_t = const.tile([2, 1], F32)
            nc.sync.dma_start(b2g_t[:], b2g_d[:])

            # masks / lengths
            len1_t = const.tile([nb, 1], I32)
            nc.sync.dma_start(len1_t[:], len1_d[:])
            len2_t = const.tile([nb, 1], I32)
            nc.sync.dma_start(len2_t[:], len2_d[:])
            iota_t = const.tile([nb, L], I32)
            nc.gpsimd.iota(iota_t[:], pattern=[[1, L]], base=0, channel_multiplier=0)

            lmT = []          # transposed logmasks: per sentence, 2 tiles [128, nb] f32
            for s, lent in ((0, len1_t), (1, len2_t)):
                m = const.tile([nb, L], F32, name=f"mask{s}", tag=f"mask{s}")
                nc.vector.tensor_tensor(m[:], iota_t[:], lent[:].to_broadcast([nb, L]), op=ALU.is_lt)
                lm = const.tile([nb, L], F32, name=f"lm{s}", tag=f"lm{s}")
                nc.vector.tensor_scalar(lm[:], m[:], 1.0, 30000.0, op0=ALU.subtract, op1=ALU.mult)
                lts = []
                for c in range(2):
                    tp = psH.tile([P, nb], F32, name="lmT_ps", tag="h")
                    nc.tensor.transpose(tp[:], lm[:, c * P:(c + 1) * P], ident[:nb, :nb])
                    lt = const.tile([P, nb], F32, name=f"lmT{s}{c}", tag=f"lmT{s}{c}")
                    nc.vector.tensor_copy(lt[:], tp[:])
                    lts.append(lt)
                lmT.append(lts)

            # per-batch masks are built as tiny [1, L] rows from iota_row + len_f
            len_f = []
            for s, ld in ((0, len1_d), (1, len2_d)):
                lf = const.tile([1, nb], I32, name=f"lenf{s}", tag=f"lenf{s}")
                nc.sync.dma_start(lf[:], ld[:].rearrange("n one -> one n"))
                len_f.append(lf)
            iota_row = const.tile([1, L], I32)
            nc.gpsimd.iota(iota_row[:], pattern=[[1, L]], base=0, channel_multiplier=0)

            # token indices, transposed to [128, nb] int32 per chunk
            sT = []
            for s, sd in ((0, s1_d), (1, s2_d)):
                st = const.tile([nb, L], I32, name=f"s{s}", tag=f"s{s}")
                nc.sync.dma_start(st[:], sd[:])
                sf = const.tile([nb, L], F32, name=f"sf{s}", tag=f"sf{s}")
                nc.vector.tensor_copy(sf[:], st[:])
                chunks = []
                for c in range(2):
                    tp = psH.tile([P, nb], F32, name="sT_ps", tag="h")
                    nc.tensor.transpose(tp[:], sf[:, c * P:(c + 1) * P], ident[:nb, :nb])
                    tf = const.tile([P, nb], F32, name=f"sTf{s}{c}", tag=f"sTf{s}{c}")
                    nc.vector.tensor_copy(tf[:], tp[:])
                    ti = const.tile([P, nb], I32, name=f"sTi{s}{c}", tag=f"sTi{s}{c}")
                    nc.vector.tensor_copy(ti[:], tf[:])
                    chunks.append(ti)
                sT.append(chunks)

            # v accumulators [100, nb] per H-chunk per sentence
            v_all = [[const.tile([100, nb], F32, name=f"v{s}{m}", tag=f"v{s}{m}") for m in range(2)] for s in range(2)]

            # ---------------- per-batch loop ----------------
            # Both sentences are CONCATENATED along the free dim (cols s*L..)
            # so every shared-weight MLP matmul runs at N=2L=512: half the
            # matmul + LDWEIGHTS + activation instruction count.
            L2 = 2 * L
            for b in range(nb):
                # concatenated logmask row [1, 2L] bf16: 0 at valid positions,
                # -30000 at padded ones.  Folded into the compare-l2 matmul as a
                # K=1 row so relu emits exact 0 at padded columns (replaces the
                # maskbc broadcast + scr multiply).
                lm_row = sm.tile([1, L2], BF16, name="lmrow", tag="lmrow")
                for s in range(2):
                    nc.vector.tensor_tensor(lm_row[:, s * L:(s + 1) * L], iota_row[:],
                                            len_f[s][:, b:b + 1].to_broadcast([1, L]), op=ALU.is_lt)
                nc.vector.tensor_scalar(lm_row[:], lm_row[:], 1.0, 30000.0,
                                        op0=ALU.subtract, op1=ALU.mult)
                # bf16 natural [128, 301] x2 chunks per sentence; col 300 is a
                # constant 1.0 so the m3=2 attention-sum matmul also emits the
                # softmax denominator (column sums of u) as output row 100.
                eR = [[], []]
                for s in range(2):
                    for c in range(2):
                        en = gat.tile([P, EMBED + 1], BF16, name=f"eN{s}{c}", tag=f"eN{s}{c}")
                        nc.gpsimd.indirect_dma_start(
                            out=en[:, 0:EMBED], out_offset=None, in_=emb_d[:],
                            in_offset=bass.IndirectOffsetOnAxis(ap=sT[s][c][:, b:b + 1], axis=0),
                        )
                        nc.vector.memset(en[:, EMBED:EMBED + 1], 1.0)
                        eR[s].append(en)
                # eT2[k]: [100, 2L] transposed embeddings, col block s*L + c*P.
                # Both c-chunk transposes land in one PSUM tile -> one copy.
                eT2 = [eTp.tile([100, L2], BF16, name=f"eT{k}", tag=f"eT{k}") for k in range(3)]
                for s in range(2):
                    for k, (k0, k1) in enumerate(EK):
                        tp = psH.tile([P, L], BF16, name="tr_ps", tag="h")
                        for c in range(2):
                            nc.tensor.transpose(tp[:100, c * P:(c + 1) * P], eR[s][c][:, k0:k1], ident_b[:])
                        nc.vector.tensor_copy(eT2[k][:, s * L:(s + 1) * L], tp[:100, :])
                # attend MLP over both sentences at once (N=512)
                ha2 = []
                for m, (m0, m1) in enumerate(H2):
                    pp = psH.tile([100, L2], F32, name="h1_ps", tag="h")
                    for k in range(3):
                        nc.tensor.matmul(pp[:], W1a_t[k][:, m0:m1], eT2[k][:],
                                         start=(k == 0), stop=(k == 2))
                    h = hp.tile([100, L2], BF16, name=f"ha{m}", tag=f"ha{m}")
                    nc.scalar.activation(h[:], pp[:], ACTF.Relu, bias=b1a_t[m][:], scale=1.0)
                    ha2.append(h)
                hT2 = []
                for m, (m0, m1) in enumerate(H2):
                    qp = psH.tile([100, L2], F32, name="h2_ps", tag="h")
                    for k2 in range(2):
                        nc.tensor.matmul(qp[:], W2a_t[k2][:, m0:m1], ha2[k2][:],
                                         start=(k2 == 0), stop=(k2 == 1))
                    h = hp.tile([100, L2], BF16, name=f"hT{m}", tag=f"hT{m}")
                    nc.scalar.activation(h[:], qp[:], ACTF.Relu, bias=b2a_t[m][:], scale=1.0)
                    hT2.append(h)

                # scores: e [i, j] and e^T [j, i]; stay in PSUM (rowmax, subtract
                # and exp all read PSUM directly - no SBUF staging copy)
                e_ps, eT_ps = [], []
                for ic in range(2):
                    ep = psS.tile([P, L], F32, name=f"e_ps{ic}", tag="score")
                    for m in range(2):
                        nc.tensor.matmul(ep[:], hT2[m][:, ic * P:(ic + 1) * P], hT2[m][:, L:L2],
                                         start=(m == 0), stop=(m == 1))
                    e_ps.append(ep)
                for jc in range(2):
                    ep = psS.tile([P, L], F32, name=f"eT_ps{jc}", tag="score")
                    for m in range(2):
                        nc.tensor.matmul(ep[:], hT2[m][:, L + jc * P:L + (jc + 1) * P], hT2[m][:, 0:L],
                                         start=(m == 0), stop=(m == 1))
                    eT_ps.append(ep)

                # exp(e) * mask1[i];  exp(eT) * mask2[j].
                # The reference's max-subtraction is skipped entirely: any shared
                # M cancels exactly between numerator and denominator, and here
                # e is bounded (~|e| < 10) so exp cannot overflow.  ACT reads the
                # score PSUM directly.
                u = [[], []]  # u[0]=uA (i-part), u[1]=uB (j-part)
                for d, (eps, lmTs) in enumerate(((e_ps, lmT[0]), (eT_ps, lmT[1]))):
                    for c in range(2):
                        uu = sm.tile([P, L], BF16, name=f"u{d}{c}", tag=f"u{d}{c}")
                        nc.scalar.activation(uu[:], eps[c][:], ACTF.Exp, bias=lmTs[c][:, b:b + 1], scale=1.0)
                        u[d].append(uu)

                # attention sums: alphas^T = e1^T-chunks @ uA, betas^T = e2-chunks @ uB.
                # Written into xT2 col block (1-d)*L so block 0 = betasT (pairs
                # with sentence 0) and block L = alphasT (pairs with sentence 1).
                # The m3=2 chunk uses the ones-augmented lhsT [128, 101]: its
                # output row 100 is the softmax denominator, reciprocal'd on ACT
                # and partition-broadcast on GpSimd into the normalizer R.
                xT2 = [att.tile([k1 - k0, L2], BF16, name=f"xT{m3}", tag=f"xT{m3}") for m3, (k0, k1) in enumerate(E3)]
                for d in range(2):
                    ap2 = psA.tile([97, L], F32, name="attn_ps2", tag="mm")
                    for c in range(2):
                        nc.tensor.matmul(ap2[:], eR[d][c][:, 204:EMBED + 1], u[d][c][:],
                                         start=(c == 0), stop=(c == 1))
                    rlog = sm.tile([1, L], F32, name=f"rlog{d}", tag=f"rlog{d}")
                    nc.scalar.activation(rlog[:], ap2[96:97, :], ACTF.Ln, bias=0.0, scale=1.0)
                    rrow = sm.tile([1, L], F32, name=f"rrow{d}", tag=f"rrow{d}")
                    nc.scalar.activation(rrow[:], rlog[:], ACTF.Exp, bias=0.0, scale=-1.0)
                    rb = sm.tile([P, L], F32, name=f"R_bc{d}", tag=f"R_bc{d}")
                    nc.gpsimd.partition_broadcast(rb[:], rrow[:])
                    nc.vector.tensor_tensor(xT2[2][:, (1 - d) * L:(2 - d) * L], ap2[:96, :],
                                            rb[:96, :], op=ALU.mult)
                    for m3 in range(2):
                        m0, m1 = E3[m3]
                        ap_ = psA.tile([m1 - m0, L], F32, name="attn_ps", tag="mm")
                        for c in range(2):
                            nc.tensor.matmul(ap_[:], eR[d][c][:, m0:m1], u[d][c][:],
                                             start=(c == 0), stop=(c == 1))
                        nc.vector.tensor_tensor(xT2[m3][:, (1 - d) * L:(2 - d) * L], ap_[:],
                                                rb[:m1 - m0, :], op=ALU.mult)

                # compare MLP over both sentences at once (N=512) + masked sum
                r1 = []
                for m, (m0, m1) in enumerate(H2):
                    up = psA.tile([100, L2], F32, name="c1_ps", tag="mm")
                    for k in range(3):
                        nc.tensor.matmul(up[:], W1ca_t[k][:, m0:m1], eT2[k][:],
                                         start=(k == 0), stop=False)
                    for k3 in range(3):
                        nc.tensor.matmul(up[:], W1cb_t[k3][:, m0:m1], xT2[k3][:],
                                         start=False, stop=(k3 == 2))
                    r = cmp_.tile([100, L2], BF16, name=f"r1{m}", tag=f"r1{m}")
                    nc.scalar.activation(r[:], up[:], ACTF.Relu, bias=b1c_t[m][:], scale=1.0)
                    r1.append(r)
                for m, (m0, m1) in enumerate(H2):
                    cp = psA.tile([100, L2], F32, name="c2_ps", tag="mm")
                    for k2 in range(2):
                        nc.tensor.matmul(cp[:], W2c_t[k2][:, m0:m1], r1[k2][:],
                                         start=(k2 == 0), stop=False)
                    # fold -30000 into padded columns so relu outputs exact 0
                    # there: the masked sum becomes a plain reduce.
                    nc.tensor.matmul(cp[:], ones_row_b[:, :100], lm_row[:],
                                     start=False, stop=True)
                    c2 = cmp_.tile([100, L2], F32, name=f"c2{m}", tag=f"c2{m}")
                    nc.scalar.activation(c2[:], cp[:], ACTF.Relu, bias=b2c_t[m][:], scale=1.0)
                    for s in range(2):
                        nc.vector.tensor_reduce(v_all[s][m][:, b:b + 1], c2[:, s * L:(s + 1) * L],
                                                axis=AX.X, op=ALU.add)

            # ---------------- aggregate ----------------
            vr = []
            for s in range(2):
                for m in range(2):
                    t = const.tile([100, nb], BF16, name=f"vr{s}{m}", tag=f"vr{s}{m}")
                    nc.vector.tensor_copy(t[:], v_all[s][m][:])
                    vr.append(t)
            if DBG:
                for i in range(4):
                    nc.sync.dma_start(vdbg_d[i * 100:(i + 1) * 100, :], v_all[i // 2][i % 2][:])
            g1 = []
            for m, (m0, m1) in enumerate(H2):
                gp = psA.tile([100, nb], F32, name="g_ps", tag="mm")
                for k in range(4):
                    nc.tensor.matmul(gp[:], W1g_t[k][:, m0:m1], vr[k][:],
                                     start=(k == 0), stop=(k == 3))
                g = const.tile([100, nb], BF16, name=f"g1{m}", tag=f"g1{m}")
                nc.scalar.activation(g[:], gp[:], ACTF.Relu, bias=b1g_t[m][:], scale=1.0)
                g1.append(g)
            op = psA.tile([2, nb], F32, name="o_ps", tag="mm")
            for k2 in range(2):
                nc.tensor.matmul(op[:], W2g_t[k2][:], g1[k2][:],
                                 start=(k2 == 0), stop=(k2 == 1))
            osb = const.tile([2, nb], F32, name="osb", tag="osb")
            nc.scalar.activation(osb[:], op[:], ACTF.Identity, bias=b2g_t[:], scale=1.0)
            nc.sync.dma_start(out_d[:].rearrange("b o -> o b"), osb[:])

    nc.compile()
    return nc


def _shard_inputs(inputs, nb=BC, ncores=NCORES):
    import ml_dtypes
    bf16 = ml_dtypes.bfloat16
    f = np.ascontiguousarray
    emb_b = f(inputs['emb'].astype(bf16))
    Ws = {k: f(inputs[k].astype(bf16)) for k in ('W1a', 'W2a', 'W1c', 'W2c', 'W1g', 'W2g')}
    bs = {k: f(inputs[k].reshape(-1, 1).astype(np.float32))
          for k in ('b1a', 'b2a', 'b1c', 'b2c', 'b1g', 'b2g')}
    maps = []
    for c in range(ncores):
        sl = slice(c * nb, (c + 1) * nb)
        maps.append(dict(
            emb=emb_b,
            s1=f(inputs['s1'][sl].astype(np.int32)),
            s2=f(inputs['s2'][sl].astype(np.int32)),
            len1=f(inputs['len1'][sl].reshape(nb, 1).astype(np.int32)),
            len2=f(inputs['len2'][sl].reshape(nb, 1).astype(np.int32)),
            **Ws, **bs,
        ))
    return maps


def kernel(**inputs):
    from concourse.bass_utils import run_bass_kernel_spmd
    if 'prog' not in _prog_cache:
        _prog_cache['prog'] = build_program(BC)
    nc = _prog_cache['prog']
    in_maps = _shard_inputs(inputs)
    res = run_bass_kernel_spmd(nc, in_maps, core_ids=list(range(NCORES)))
    out = np.concatenate([res.results[c]["out"] for c in range(NCORES)], axis=0)
    return out.astype(np.float32)


# revision 36
# speedup vs baseline: 1.6160x; 1.6160x over previous
"""Trainium2 Bass kernel for DecomposableAttention (B=512, L=256, V=50000, E=300, H=200).

Strategy: data-parallel over batch across 8 cores (64 batches/core).  Per batch:
indirect-DMA gather of bf16 embedding rows, on-chip PE transposes to get the
E-on-partitions layout, bf16 matmuls (1 cycle/row) for the attend/compare MLPs
and the attention einsums with fp32 PSUM accumulation, ACT-exp softmaxes in
fp32 with the length masks folded in as per-partition -30000 biases, and a
final aggregate MLP over all 64 batches.  All matmul free dims are 256.

The softmax max-subtraction uses one shared rowmax M broadcast to both
directions; since the same M value multiplies numerator and denominator of
each normalized weight, its precision cancels exactly - so the broadcast can
be bf16.  Reductions/exp/normalization stay fp32.
"""
import os as _os
import sys

if '/opt/trn_rl_repo' not in sys.path:
    sys.path.insert(0, '/opt/trn_rl_repo')

import numpy as np

B, L, VOCAB, EMBED, HIDDEN = 512, 256, 50000, 300, 200
NCORES = 8
BC = B // NCORES  # batches per core

_prog_cache = {}


def build_program(nb=BC):
    import concourse.bass as bass
    import concourse.bacc as bacc
    import concourse.tile as tile
    import concourse.mybir as mybir
    from concourse.masks import make_identity

    F32 = mybir.dt.float32
    BF16 = mybir.dt.bfloat16
    I32 = mybir.dt.int32
    AX = mybir.AxisListType
    ALU = mybir.AluOpType
    ACTF = mybir.ActivationFunctionType
    P = 128
    EK = [(0, 100), (100, 200), (200, 300)]      # E contraction chunks of 100
    H2 = [(0, 100), (100, 200)]                  # H chunks of 100
    E3 = [(0, 100), (100, 200), (200, 300)]      # E output chunks of 100

    nc = bacc.Bacc("TRN2", num_devices=NCORES)

    emb_d = nc.dram_tensor("emb", [VOCAB, EMBED], BF16, kind="ExternalInput")
    s1_d = nc.dram_tensor("s1", [nb, L], I32, kind="ExternalInput")
    s2_d = nc.dram_tensor("s2", [nb, L], I32, kind="ExternalInput")
    len1_d = nc.dram_tensor("len1", [nb, 1], I32, kind="ExternalInput")
    len2_d = nc.dram_tensor("len2", [nb, 1], I32, kind="ExternalInput")
    W1a_d = nc.dram_tensor("W1a", [EMBED, HIDDEN], BF16, kind="ExternalInput")
    W2a_d = nc.dram_tensor("W2a", [HIDDEN, HIDDEN], BF16, kind="ExternalInput")
    W1c_d = nc.dram_tensor("W1c", [2 * EMBED, HIDDEN], BF16, kind="ExternalInput")
    W2c_d = nc.dram_tensor("W2c", [HIDDEN, HIDDEN], BF16, kind="ExternalInput")
    W1g_d = nc.dram_tensor("W1g", [2 * HIDDEN, HIDDEN], BF16, kind="ExternalInput")
    W2g_d = nc.dram_tensor("W2g", [HIDDEN, 2], BF16, kind="ExternalInput")
    b1a_d = nc.dram_tensor("b1a", [HIDDEN, 1], F32, kind="ExternalInput")
    b2a_d = nc.dram_tensor("b2a", [HIDDEN, 1], F32, kind="ExternalInput")
    b1c_d = nc.dram_tensor("b1c", [HIDDEN, 1], F32, kind="ExternalInput")
    b2c_d = nc.dram_tensor("b2c", [HIDDEN, 1], F32, kind="ExternalInput")
    b1g_d = nc.dram_tensor("b1g", [HIDDEN, 1], F32, kind="ExternalInput")
    b2g_d = nc.dram_tensor("b2g", [2, 1], F32, kind="ExternalInput")
    out_d = nc.dram_tensor("out", [nb, 2], F32, kind="ExternalOutput")
    DBG = _os.environ.get("KDBG", "") == "1"
    if DBG:
        vdbg_d = nc.dram_tensor("vdbg", [400, nb], F32, kind="ExternalOutput")

    with tile.TileContext(nc) as tc:
        import contextlib
        ctx = contextlib.ExitStack()
        with ctx:
            const = ctx.enter_context(tc.tile_pool(name="const", bufs=1))
            psA = ctx.enter_context(tc.tile_pool(name="psA", bufs=3, space="PSUM"))
            psH = ctx.enter_context(tc.tile_pool(name="psH", bufs=2, space="PSUM"))
            psS = ctx.enter_context(tc.tile_pool(name="psS", bufs=3, space="PSUM"))
            gat = ctx.enter_context(tc.tile_pool(name="gat", bufs=4))
            eTp = ctx.enter_context(tc.tile_pool(name="eTp", bufs=3))
            hp = ctx.enter_context(tc.tile_pool(name="hp", bufs=3))
            sm = ctx.enter_context(tc.tile_pool(name="sm", bufs=3))
            att = ctx.enter_context(tc.tile_pool(name="att", bufs=3))
            cmp_ = ctx.enter_context(tc.tile_pool(name="cmp", bufs=3))

            # ---------------- constants ----------------
            ident = const.tile([P, P], F32)
            make_identity(nc, ident[:])
            ident_b = const.tile([P, P], BF16)
            nc.vector.tensor_copy(ident_b[:], ident[:])

            ones_f = const.tile([P, 1], F32)
            nc.vector.memset(ones_f[:], 1.0)
            ones_col_b = const.tile([P, 1], BF16)   # lhsT for den sums (K=128, M=1)
            nc.vector.tensor_copy(ones_col_b[:], ones_f[:])
            ones_row_f = const.tile([1, P], F32)
            nc.vector.memset(ones_row_f[:], 1.0)
            ones_row_b = const.tile([1, P], BF16)   # lhsT for bcasts (K=1, M=128)
            nc.vector.tensor_copy(ones_row_b[:], ones_row_f[:])

            # weights (already bf16 in DRAM, cast host-side)
            W1a_t = [const.tile([k1 - k0, HIDDEN], BF16, name=f"W1a{i}", tag=f"W1a{i}") for i, (k0, k1) in enumerate(EK)]
            for i, (k0, k1) in enumerate(EK):
                nc.sync.dma_start(W1a_t[i][:], W1a_d[k0:k1, :])
            W2a_t = [const.tile([100, HIDDEN], BF16, name=f"W2a{i}", tag=f"W2a{i}") for i in range(2)]
            for i, (k0, k1) in enumerate(H2):
                nc.sync.dma_start(W2a_t[i][:], W2a_d[k0:k1, :])
            W1ca_t = [const.tile([k1 - k0, HIDDEN], BF16, name=f"W1ca{i}", tag=f"W1ca{i}") for i, (k0, k1) in enumerate(EK)]
            for i, (k0, k1) in enumerate(EK):
                nc.sync.dma_start(W1ca_t[i][:], W1c_d[k0:k1, :])
            W1cb_t = [const.tile([k1 - k0, HIDDEN], BF16, name=f"W1cb{i}", tag=f"W1cb{i}") for i, (k0, k1) in enumerate(E3)]
            for i, (k0, k1) in enumerate(E3):
                nc.sync.dma_start(W1cb_t[i][:], W1c_d[EMBED + k0:EMBED + k1, :])
            W2c_t = [const.tile([100, HIDDEN], BF16, name=f"W2c{i}", tag=f"W2c{i}") for i in range(2)]
            for i, (k0, k1) in enumerate(H2):
                nc.sync.dma_start(W2c_t[i][:], W2c_d[k0:k1, :])
            W1g_t = [const.tile([100, HIDDEN], BF16, name=f"W1g{i}", tag=f"W1g{i}") for i in range(4)]
            for i in range(4):
                nc.sync.dma_start(W1g_t[i][:], W1g_d[i * 100:(i + 1) * 100, :])
            W2g_t = [const.tile([100, 2], BF16, name=f"W2g{i}", tag=f"W2g{i}") for i in range(2)]
            for i, (k0, k1) in enumerate(H2):
                nc.sync.dma_start(W2g_t[i][:], W2g_d[k0:k1, :])

            def bias2(d):
                t = [const.tile([100, 1], F32, name=f"b{d.name}{i}", tag=f"b{d.name}{i}") for i in range(2)]
                for i, (k0, k1) in enumerate(H2):
                    nc.sync.dma_start(t[i][:], d[k0:k1, :])
                return t
            b1a_t, b2a_t = bias2(b1a_d), bias2(b2a_d)
            b1c_t, b2c_t = bias2(b1c_d), bias2(b2c_d)
            b1g_t = bias2(b1g_d)
            b2g_t = const.tile([2, 1], F32)
            nc.sync.dma_start(b2g_t[:], b2g_d[:])

            # masks / lengths
            len1_t = const.tile([nb, 1], I32)
            nc.sync.dma_start(len1_t[:], len1_d[:])
            len2_t = const.tile([nb, 1], I32)
            nc.sync.dma_start(len2_t[:], len2_d[:])
            iota_t = const.tile([nb, L], I32)
            nc.gpsimd.iota(iota_t[:], pattern=[[1, L]], base=0, channel_multiplier=0)

            lmT = []          # transposed logmasks: per sentence, 2 tiles [128, nb] f32
            for s, lent in ((0, len1_t), (1, len2_t)):
                m = const.tile([nb, L], F32, name=f"mask{s}", tag=f"mask{s}")
                nc.vector.tensor_tensor(m[:], iota_t[:], lent[:].to_broadcast([nb, L]), op=ALU.is_lt)
                lm = const.tile([nb, L], F32, name=f"lm{s}", tag=f"lm{s}")
                nc.vector.tensor_scalar(lm[:], m[:], 1.0, 30000.0, op0=ALU.subtract, op1=ALU.mult)
                lts = []
                for c in range(2):
                    tp = psH.tile([P, nb], F32, name="lmT_ps", tag="h")
                    nc.tensor.transpose(tp[:], lm[:, c * P:(c + 1) * P], ident[:nb, :nb])
                    lt = const.tile([P, nb], F32, name=f"lmT{s}{c}", tag=f"lmT{s}{c}")
                    nc.vector.tensor_copy(lt[:], tp[:])
                    lts.append(lt)
                lmT.append(lts)

            # per-batch masks are built as tiny [1, L] rows from iota_row + len_f
            len_f = []
            for s, ld in ((0, len1_d), (1, len2_d)):
                lf = const.tile([1, nb], I32, name=f"lenf{s}", tag=f"lenf{s}")
                nc.sync.dma_start(lf[:], ld[:].rearrange("n one -> one n"))
                len_f.append(lf)
            iota_row = const.tile([1, L], I32)
            nc.gpsimd.iota(iota_row[:], pattern=[[1, L]], base=0, channel_multiplier=0)

            # token indices, transposed to [128, nb] int32 per chunk
            sT = []
            for s, sd in ((0, s1_d), (1, s2_d)):
                st = const.tile([nb, L], I32, name=f"s{s}", tag=f"s{s}")
                nc.sync.dma_start(st[:], sd[:])
                sf = const.tile([nb, L], F32, name=f"sf{s}", tag=f"sf{s}")
                nc.vector.tensor_copy(sf[:], st[:])
                chunks = []
                for c in range(2):
                    tp = psH.tile([P, nb], F32, name="sT_ps", tag="h")
                    nc.tensor.transpose(tp[:], sf[:, c * P:(c + 1) * P], ident[:nb, :nb])
                    tf = const.tile([P, nb], F32, name=f"sTf{s}{c}", tag=f"sTf{s}{c}")
                    nc.vector.tensor_copy(tf[:], tp[:])
                    ti = const.tile([P, nb], I32, name=f"sTi{s}{c}", tag=f"sTi{s}{c}")
                    nc.vector.tensor_copy(ti[:], tf[:])
                    chunks.append(ti)
                sT.append(chunks)

            # v accumulators [100, nb] per H-chunk per sentence
            v_all = [[const.tile([100, nb], F32, name=f"v{s}{m}", tag=f"v{s}{m}") for m in range(2)] for s in range(2)]

            # ---------------- per-batch loop ----------------
            # Both sentences are CONCATENATED along the free dim (cols s*L..)
            # so every shared-weight MLP matmul runs at N=2L=512: half the
            # matmul + LDWEIGHTS + activation instruction count.
            L2 = 2 * L
            for b in range(nb):
                # concatenated logmask row [1, 2L] bf16: 0 at valid positions,
                # -30000 at padded ones.  Folded into the compare-l2 matmul as a
                # K=1 row so relu emits exact 0 at padded columns (replaces the
                # maskbc broadcast + scr multiply).
                lm_row = sm.tile([1, L2], BF16, name="lmrow", tag="lmrow")
                for s in range(2):
                    nc.vector.tensor_tensor(lm_row[:, s * L:(s + 1) * L], iota_row[:],
                                            len_f[s][:, b:b + 1].to_broadcast([1, L]), op=ALU.is_lt)
                nc.vector.tensor_scalar(lm_row[:], lm_row[:], 1.0, 30000.0,
                                        op0=ALU.subtract, op1=ALU.mult)
                eR = [[], []]   # bf16 natural [128, 300] x2 chunks per sentence
                for s in range(2):
                    for c in range(2):
                        en = gat.tile([P, EMBED], BF16, name=f"eN{s}{c}", tag=f"eN{s}{c}")
                        nc.gpsimd.indirect_dma_start(
                            out=en[:], out_offset=None, in_=emb_d[:],
                            in_offset=bass.IndirectOffsetOnAxis(ap=sT[s][c][:, b:b + 1], axis=0),
                        )
                        eR[s].append(en)
                # eT2[k]: [100, 2L] transposed embeddings, col block s*L + c*P.
                # Both c-chunk transposes land in one PSUM tile -> one copy.
                eT2 = [eTp.tile([100, L2], BF16, name=f"eT{k}", tag=f"eT{k}") for k in range(3)]
                for s in range(2):
                    for k, (k0, k1) in enumerate(EK):
                        tp = psH.tile([P, L], BF16, name="tr_ps", tag="h")
                        for c in range(2):
                            nc.tensor.transpose(tp[:100, c * P:(c + 1) * P], eR[s][c][:, k0:k1], ident_b[:])
                        nc.vector.tensor_copy(eT2[k][:, s * L:(s + 1) * L], tp[:100, :])
                # attend MLP over both sentences at once (N=512)
                ha2 = []
                for m, (m0, m1) in enumerate(H2):
                    pp = psH.tile([100, L2], F32, name="h1_ps", tag="h")
                    for k in range(3):
                        nc.tensor.matmul(pp[:], W1a_t[k][:, m0:m1], eT2[k][:],
                                         start=(k == 0), stop=(k == 2))
                    h = hp.tile([100, L2], BF16, name=f"ha{m}", tag=f"ha{m}")
                    nc.scalar.activation(h[:], pp[:], ACTF.Relu, bias=b1a_t[m][:], scale=1.0)
                    ha2.append(h)
                hT2 = []
                for m, (m0, m1) in enumerate(H2):
                    qp = psH.tile([100, L2], F32, name="h2_ps", tag="h")
                    for k2 in range(2):
                        nc.tensor.matmul(qp[:], W2a_t[k2][:, m0:m1], ha2[k2][:],
                                         start=(k2 == 0), stop=(k2 == 1))
                    h = hp.tile([100, L2], BF16, name=f"hT{m}", tag=f"hT{m}")
                    nc.scalar.activation(h[:], qp[:], ACTF.Relu, bias=b2a_t[m][:], scale=1.0)
                    hT2.append(h)

                # scores: e [i, j] and e^T [j, i]; stay in PSUM (rowmax, subtract
                # and exp all read PSUM directly - no SBUF staging copy)
                e_ps, eT_ps = [], []
                for ic in range(2):
                    ep = psS.tile([P, L], F32, name=f"e_ps{ic}", tag="score")
                    for m in range(2):
                        nc.tensor.matmul(ep[:], hT2[m][:, ic * P:(ic + 1) * P], hT2[m][:, L:L2],
                                         start=(m == 0), stop=(m == 1))
                    e_ps.append(ep)
                for jc in range(2):
                    ep = psS.tile([P, L], F32, name=f"eT_ps{jc}", tag="score")
                    for m in range(2):
                        nc.tensor.matmul(ep[:], hT2[m][:, L + jc * P:L + (jc + 1) * P], hT2[m][:, 0:L],
                                         start=(m == 0), stop=(m == 1))
                    eT_ps.append(ep)

                # exp(e) * mask1[i];  exp(eT) * mask2[j].
                # The reference's max-subtraction is skipped entirely: any shared
                # M cancels exactly between numerator and denominator, and here
                # e is bounded (~|e| < 10) so exp cannot overflow.  ACT reads the
                # score PSUM directly.
                u = [[], []]  # u[0]=uA (i-part), u[1]=uB (j-part)
                for d, (eps, lmTs) in enumerate(((e_ps, lmT[0]), (eT_ps, lmT[1]))):
                    for c in range(2):
                        uu = sm.tile([P, L], BF16, name=f"u{d}{c}", tag=f"u{d}{c}")
                        nc.scalar.activation(uu[:], eps[c][:], ACTF.Exp, bias=lmTs[c][:, b:b + 1], scale=1.0)
                        u[d].append(uu)

                # attention sums: alphas^T = e1^T-chunks @ uA, betas^T = e2-chunks @ uB.
                # Written into xT2 col block (1-d)*L so block 0 = betasT (pairs
                # with sentence 0) and block L = alphasT (pairs with sentence 1).
                # The m3=2 chunk uses the ones-augmented lhsT [128, 101]: its
                # output row 100 is the softmax denominator, reciprocal'd on ACT
                # and partition-broadcast on GpSimd into the normalizer R.
                xT2 = [att.tile([100, L2], BF16, name=f"xT{m3}", tag=f"xT{m3}") for m3 in range(3)]
                for d in range(2):
                    dpT = psA.tile([P, 2], F32, name=f"denT_ps{d}", tag="mm")
                    for jc in range(2):
                        for c in range(2):
                            nc.tensor.matmul(dpT[:, jc:jc + 1], u[d][c][:, jc * P:(jc + 1) * P],
                                             ones_col_b[:], start=(c == 0), stop=(c == 1))
                    rinvT = sm.tile([P, 2], BF16, name=f"rinvT{d}", tag=f"rinvT{d}")
                    with nc.allow_low_precision(reason="bf16 reciprocal; 0.4% rel err acceptable"):
                        nc.vector.reciprocal(rinvT[:], dpT[:])
                    rrow_ps = psA.tile([1, L], BF16, name=f"rrow_ps{d}", tag="mm")
                    with nc.allow_low_precision(reason="bf16 PE transpose of reciprocals; no accumulation"):
                        for jc in range(2):
                            nc.tensor.transpose(rrow_ps[:, jc * P:(jc + 1) * P],
                                                rinvT[:, jc:jc + 1], ident_b[:])
                    rrow = sm.tile([1, L], F32, name=f"rrow{d}", tag=f"rrow{d}")
                    nc.any.tensor_copy(rrow[:], rrow_ps[:])
                    rb = sm.tile([P, L], F32, name=f"R_bc{d}", tag=f"R_bc{d}")
                    nc.gpsimd.partition_broadcast(rb[:], rrow[:])
                    for m3 in range(3):
                        m0, m1 = E3[m3]
                        ap_ = psA.tile([100, L], F32, name="attn_ps", tag="mm")
                        for c in range(2):
                            nc.tensor.matmul(ap_[:], eR[d][c][:, m0:m1], u[d][c][:],
                                             start=(c == 0), stop=(c == 1))
                        nc.vector.tensor_tensor(xT2[m3][:, (1 - d) * L:(2 - d) * L], ap_[:],
                                                rb[:100, :], op=ALU.mult)

                # compare MLP over both sentences at once (N=512) + masked sum
                r1 = []
                for m, (m0, m1) in enumerate(H2):
                    up = psA.tile([100, L2], F32, name="c1_ps", tag="mm")
                    for k in range(3):
                        nc.tensor.matmul(up[:], W1ca_t[k][:, m0:m1], eT2[k][:],
                                         start=(k == 0), stop=False)
                    for k3 in range(3):
                        nc.tensor.matmul(up[:], W1cb_t[k3][:, m0:m1], xT2[k3][:],
                                         start=False, stop=(k3 == 2))
                    r = cmp_.tile([100, L2], BF16, name=f"r1{m}", tag=f"r1{m}")
                    nc.scalar.activation(r[:], up[:], ACTF.Relu, bias=b1c_t[m][:], scale=1.0)
                    r1.append(r)
                for m, (m0, m1) in enumerate(H2):
                    cp = psA.tile([100, L2], F32, name="c2_ps", tag="mm")
                    for k2 in range(2):
                        nc.tensor.matmul(cp[:], W2c_t[k2][:, m0:m1], r1[k2][:],
                                         start=(k2 == 0), stop=False)
                    # fold -30000 into padded columns so relu outputs exact 0
                    # there: the masked sum becomes a plain reduce.
                    nc.tensor.matmul(cp[:], ones_row_b[:, :100], lm_row[:],
                                     start=False, stop=True)
                    c2 = cmp_.tile([100, L2], F32, name=f"c2{m}", tag=f"c2{m}")
                    nc.scalar.activation(c2[:], cp[:], ACTF.Relu, bias=b2c_t[m][:], scale=1.0)
                    for s in range(2):
                        nc.vector.tensor_reduce(v_all[s][m][:, b:b + 1], c2[:, s * L:(s + 1) * L],
                                                axis=AX.X, op=ALU.add)

            # ---------------- aggregate ----------------
            vr = []
            for s in range(2):
                for m in range(2):
                    t = const.tile([100, nb], BF16, name=f"vr{s}{m}", tag=f"vr{s}{m}")
                    nc.vector.tensor_copy(t[:], v_all[s][m][:])
                    vr.append(t)
            if DBG:
                for i in range(4):
                    nc.sync.dma_start(vdbg_d[i * 100:(i + 1) * 100, :], v_all[i // 2][i % 2][:])
            g1 = []
            for m, (m0, m1) in enumerate(H2):
                gp = psA.tile([100, nb], F32, name="g_ps", tag="mm")
                for k in range(4):
                    nc.tensor.matmul(gp[:], W1g_t[k][:, m0:m1], vr[k][:],
                                     start=(k == 0), stop=(k == 3))
                g = const.tile([100, nb], BF16, name=f"g1{m}", tag=f"g1{m}")
                nc.scalar.activation(g[:], gp[:], ACTF.Relu, bias=b1g_t[m][:], scale=1.0)
                g1.append(g)
            op = psA.tile([2, nb], F32, name="o_ps", tag="mm")
            for k2 in range(2):
                nc.tensor.matmul(op[:], W2g_t[k2][:], g1[k2][:],
                                 start=(k2 == 0), stop=(k2 == 1))
            osb = const.tile([2, nb], F32, name="osb", tag="osb")
            nc.scalar.activation(osb[:], op[:], ACTF.Identity, bias=b2g_t[:], scale=1.0)
            nc.sync.dma_start(out_d[:].rearrange("b o -> o b"), osb[:])

    nc.compile()
    return nc


def _shard_inputs(inputs, nb=BC, ncores=NCORES):
    import ml_dtypes
    bf16 = ml_dtypes.bfloat16
    f = np.ascontiguousarray
    emb_b = f(inputs['emb'].astype(bf16))
    Ws = {k: f(inputs[k].astype(bf16)) for k in ('W1a', 'W2a', 'W1c', 'W2c', 'W1g', 'W2g')}
    bs = {k: f(inputs[k].reshape(-1, 1).astype(np.float32))
          for k in ('b1a', 'b2a', 'b1c', 'b2c', 'b1g', 'b2g')}
    maps = []
    for c in range(ncores):
        sl = slice(c * nb, (c + 1) * nb)
        maps.append(dict(
            emb=emb_b,
            s1=f(inputs['s1'][sl].astype(np.int32)),
            s2=f(inputs['s2'][sl].astype(np.int32)),
            len1=f(inputs['len1'][sl].reshape(nb, 1).astype(np.int32)),
            len2=f(inputs['len2'][sl].reshape(nb, 1).astype(np.int32)),
            **Ws, **bs,
        ))
    return maps


def kernel(**inputs):
    from concourse.bass_utils import run_bass_kernel_spmd
    if 'prog' not in _prog_cache:
        _prog_cache['prog'] = build_program(BC)
    nc = _prog_cache['prog']
    in_maps = _shard_inputs(inputs)
    res = run_bass_kernel_spmd(nc, in_maps, core_ids=list(range(NCORES)))
    out = np.concatenate([res.results[c]["out"] for c in range(NCORES)], axis=0)
    return out.astype(np.float32)


# revision 37
# speedup vs baseline: 1.8402x; 1.1387x over previous
"""Trainium2 Bass kernel for DecomposableAttention (B=512, L=256, V=50000, E=300, H=200).

Strategy: data-parallel over batch across 8 cores (64 batches/core).  Per batch:
indirect-DMA gather of bf16 embedding rows, on-chip PE transposes to get the
E-on-partitions layout, bf16 matmuls (1 cycle/row) for the attend/compare MLPs
and the attention einsums with fp32 PSUM accumulation, ACT-exp softmaxes in
fp32 with the length masks folded in as per-partition -30000 biases, and a
final aggregate MLP over all 64 batches.  All matmul free dims are 256.

The softmax max-subtraction uses one shared rowmax M broadcast to both
directions; since the same M value multiplies numerator and denominator of
each normalized weight, its precision cancels exactly - so the broadcast can
be bf16.  Reductions/exp/normalization stay fp32.
"""
import os as _os
import sys

if '/opt/trn_rl_repo' not in sys.path:
    sys.path.insert(0, '/opt/trn_rl_repo')

import numpy as np

B, L, VOCAB, EMBED, HIDDEN = 512, 256, 50000, 300, 200
NCORES = 8
BC = B // NCORES  # batches per core

_prog_cache = {}


def build_program(nb=BC):
    import concourse.bass as bass
    import concourse.bacc as bacc
    import concourse.tile as tile
    import concourse.mybir as mybir
    from concourse.masks import make_identity

    F32 = mybir.dt.float32
    BF16 = mybir.dt.bfloat16
    I32 = mybir.dt.int32
    AX = mybir.AxisListType
    ALU = mybir.AluOpType
    ACTF = mybir.ActivationFunctionType
    P = 128
    EK = [(0, 100), (100, 200), (200, 300)]      # E contraction chunks of 100
    H2 = [(0, 100), (100, 200)]                  # H chunks of 100
    E3 = [(0, 100), (100, 200), (200, 300)]      # E output chunks of 100

    nc = bacc.Bacc("TRN2", num_devices=NCORES)

    emb_d = nc.dram_tensor("emb", [VOCAB, EMBED], BF16, kind="ExternalInput")
    s1_d = nc.dram_tensor("s1", [nb, L], I32, kind="ExternalInput")
    s2_d = nc.dram_tensor("s2", [nb, L], I32, kind="ExternalInput")
    len1_d = nc.dram_tensor("len1", [nb, 1], I32, kind="ExternalInput")
    len2_d = nc.dram_tensor("len2", [nb, 1], I32, kind="ExternalInput")
    W1a_d = nc.dram_tensor("W1a", [EMBED, HIDDEN], BF16, kind="ExternalInput")
    W2a_d = nc.dram_tensor("W2a", [HIDDEN, HIDDEN], BF16, kind="ExternalInput")
    W1c_d = nc.dram_tensor("W1c", [2 * EMBED, HIDDEN], BF16, kind="ExternalInput")
    W2c_d = nc.dram_tensor("W2c", [HIDDEN, HIDDEN], BF16, kind="ExternalInput")
    W1g_d = nc.dram_tensor("W1g", [2 * HIDDEN, HIDDEN], BF16, kind="ExternalInput")
    W2g_d = nc.dram_tensor("W2g", [HIDDEN, 2], BF16, kind="ExternalInput")
    b1a_d = nc.dram_tensor("b1a", [HIDDEN, 1], F32, kind="ExternalInput")
    b2a_d = nc.dram_tensor("b2a", [HIDDEN, 1], F32, kind="ExternalInput")
    b1c_d = nc.dram_tensor("b1c", [HIDDEN, 1], F32, kind="ExternalInput")
    b2c_d = nc.dram_tensor("b2c", [HIDDEN, 1], F32, kind="ExternalInput")
    b1g_d = nc.dram_tensor("b1g", [HIDDEN, 1], F32, kind="ExternalInput")
    b2g_d = nc.dram_tensor("b2g", [2, 1], F32, kind="ExternalInput")
    out_d = nc.dram_tensor("out", [nb, 2], F32, kind="ExternalOutput")
    DBG = _os.environ.get("KDBG", "") == "1"
    if DBG:
        vdbg_d = nc.dram_tensor("vdbg", [400, nb], F32, kind="ExternalOutput")

    with tile.TileContext(nc) as tc:
        import contextlib
        ctx = contextlib.ExitStack()
        with ctx:
            const = ctx.enter_context(tc.tile_pool(name="const", bufs=1))
            psA = ctx.enter_context(tc.tile_pool(name="psA", bufs=3, space="PSUM"))
            psH = ctx.enter_context(tc.tile_pool(name="psH", bufs=2, space="PSUM"))
            psS = ctx.enter_context(tc.tile_pool(name="psS", bufs=3, space="PSUM"))
            gat = ctx.enter_context(tc.tile_pool(name="gat", bufs=4))
            eTp = ctx.enter_context(tc.tile_pool(name="eTp", bufs=3))
            hp = ctx.enter_context(tc.tile_pool(name="hp", bufs=3))
            sm = ctx.enter_context(tc.tile_pool(name="sm", bufs=3))
            att = ctx.enter_context(tc.tile_pool(name="att", bufs=3))
            cmp_ = ctx.enter_context(tc.tile_pool(name="cmp", bufs=3))

            # ---------------- constants ----------------
            ident = const.tile([P, P], F32)
            make_identity(nc, ident[:])
            ident_b = const.tile([P, P], BF16)
            nc.vector.tensor_copy(ident_b[:], ident[:])

            ones_f = const.tile([P, 1], F32)
            nc.vector.memset(ones_f[:], 1.0)
            ones_col_b = const.tile([P, 1], BF16)   # lhsT for den sums (K=128, M=1)
            nc.vector.tensor_copy(ones_col_b[:], ones_f[:])
            ones_row_f = const.tile([1, P], F32)
            nc.vector.memset(ones_row_f[:], 1.0)
            ones_row_b = const.tile([1, P], BF16)   # lhsT for bcasts (K=1, M=128)
            nc.vector.tensor_copy(ones_row_b[:], ones_row_f[:])

            # weights (already bf16 in DRAM, cast host-side)
            W1a_t = [const.tile([k1 - k0, HIDDEN], BF16, name=f"W1a{i}", tag=f"W1a{i}") for i, (k0, k1) in enumerate(EK)]
            for i, (k0, k1) in enumerate(EK):
                nc.sync.dma_start(W1a_t[i][:], W1a_d[k0:k1, :])
            W2a_t = [const.tile([100, HIDDEN], BF16, name=f"W2a{i}", tag=f"W2a{i}") for i in range(2)]
            for i, (k0, k1) in enumerate(H2):
                nc.sync.dma_start(W2a_t[i][:], W2a_d[k0:k1, :])
            W1ca_t = [const.tile([k1 - k0, HIDDEN], BF16, name=f"W1ca{i}", tag=f"W1ca{i}") for i, (k0, k1) in enumerate(EK)]
            for i, (k0, k1) in enumerate(EK):
                nc.sync.dma_start(W1ca_t[i][:], W1c_d[k0:k1, :])
            W1cb_t = [const.tile([k1 - k0, HIDDEN], BF16, name=f"W1cb{i}", tag=f"W1cb{i}") for i, (k0, k1) in enumerate(E3)]
            for i, (k0, k1) in enumerate(E3):
                nc.sync.dma_start(W1cb_t[i][:], W1c_d[EMBED + k0:EMBED + k1, :])
            W2c_t = [const.tile([100, HIDDEN], BF16, name=f"W2c{i}", tag=f"W2c{i}") for i in range(2)]
            for i, (k0, k1) in enumerate(H2):
                nc.sync.dma_start(W2c_t[i][:], W2c_d[k0:k1, :])
            W1g_t = [const.tile([100, HIDDEN], BF16, name=f"W1g{i}", tag=f"W1g{i}") for i in range(4)]
            for i in range(4):
                nc.sync.dma_start(W1g_t[i][:], W1g_d[i * 100:(i + 1) * 100, :])
            W2g_t = [const.tile([100, 2], BF16, name=f"W2g{i}", tag=f"W2g{i}") for i in range(2)]
            for i, (k0, k1) in enumerate(H2):
                nc.sync.dma_start(W2g_t[i][:], W2g_d[k0:k1, :])

            def bias2(d):
                t = [const.tile([100, 1], F32, name=f"b{d.name}{i}", tag=f"b{d.name}{i}") for i in range(2)]
                for i, (k0, k1) in enumerate(H2):
                    nc.sync.dma_start(t[i][:], d[k0:k1, :])
                return t
            b1a_t, b2a_t = bias2(b1a_d), bias2(b2a_d)
            b1c_t, b2c_t = bias2(b1c_d), bias2(b2c_d)
            b1g_t = bias2(b1g_d)
            b2g_t = const.tile([2, 1], F32)
            nc.sync.dma_start(b2g_t[:], b2g_d[:])

            # masks / lengths
            len1_t = const.tile([nb, 1], I32)
            nc.sync.dma_start(len1_t[:], len1_d[:])
            len2_t = const.tile([nb, 1], I32)
            nc.sync.dma_start(len2_t[:], len2_d[:])
            iota_t = const.tile([nb, L], I32)
            nc.gpsimd.iota(iota_t[:], pattern=[[1, L]], base=0, channel_multiplier=0)

            lmT = []          # transposed logmasks: per sentence, 2 tiles [128, nb] f32
            for s, lent in ((0, len1_t), (1, len2_t)):
                m = const.tile([nb, L], F32, name=f"mask{s}", tag=f"mask{s}")
                nc.vector.tensor_tensor(m[:], iota_t[:], lent[:].to_broadcast([nb, L]), op=ALU.is_lt)
                lm = const.tile([nb, L], F32, name=f"lm{s}", tag=f"lm{s}")
                nc.vector.tensor_scalar(lm[:], m[:], 1.0, 30000.0, op0=ALU.subtract, op1=ALU.mult)
                lts = []
                for c in range(2):
                    tp = psH.tile([P, nb], F32, name="lmT_ps", tag="h")
                    nc.tensor.transpose(tp[:], lm[:, c * P:(c + 1) * P], ident[:nb, :nb])
                    lt = const.tile([P, nb], F32, name=f"lmT{s}{c}", tag=f"lmT{s}{c}")
                    nc.vector.tensor_copy(lt[:], tp[:])
                    lts.append(lt)
                lmT.append(lts)

            # per-batch masks are built as tiny [1, L] rows from iota_row + len_f
            len_f = []
            for s, ld in ((0, len1_d), (1, len2_d)):
                lf = const.tile([1, nb], I32, name=f"lenf{s}", tag=f"lenf{s}")
                nc.sync.dma_start(lf[:], ld[:].rearrange("n one -> one n"))
                len_f.append(lf)
            iota_row = const.tile([1, L], I32)
            nc.gpsimd.iota(iota_row[:], pattern=[[1, L]], base=0, channel_multiplier=0)

            # token indices, transposed to [128, nb] int32 per chunk
            sT = []
            for s, sd in ((0, s1_d), (1, s2_d)):
                st = const.tile([nb, L], I32, name=f"s{s}", tag=f"s{s}")
                nc.sync.dma_start(st[:], sd[:])
                sf = const.tile([nb, L], F32, name=f"sf{s}", tag=f"sf{s}")
                nc.vector.tensor_copy(sf[:], st[:])
                chunks = []
                for c in range(2):
                    tp = psH.tile([P, nb], F32, name="sT_ps", tag="h")
                    nc.tensor.transpose(tp[:], sf[:, c * P:(c + 1) * P], ident[:nb, :nb])
                    tf = const.tile([P, nb], F32, name=f"sTf{s}{c}", tag=f"sTf{s}{c}")
                    nc.vector.tensor_copy(tf[:], tp[:])
                    ti = const.tile([P, nb], I32, name=f"sTi{s}{c}", tag=f"sTi{s}{c}")
                    nc.vector.tensor_copy(ti[:], tf[:])
                    chunks.append(ti)
                sT.append(chunks)

            # v accumulators [100, nb] per H-chunk per sentence
            v_all = [[const.tile([100, nb], F32, name=f"v{s}{m}", tag=f"v{s}{m}") for m in range(2)] for s in range(2)]

            # ---------------- per-batch loop ----------------
            # Both sentences are CONCATENATED along the free dim (cols s*L..)
            # so every shared-weight MLP matmul runs at N=2L=512: half the
            # matmul + LDWEIGHTS + activation instruction count.
            L2 = 2 * L
            for b in range(nb):
                # concatenated logmask row [1, 2L] bf16: 0 at valid positions,
                # -30000 at padded ones.  Folded into the compare-l2 matmul as a
                # K=1 row so relu emits exact 0 at padded columns (replaces the
                # maskbc broadcast + scr multiply).
                lm_row = sm.tile([1, L2], BF16, name="lmrow", tag="lmrow")
                for s in range(2):
                    nc.vector.tensor_tensor(lm_row[:, s * L:(s + 1) * L], iota_row[:],
                                            len_f[s][:, b:b + 1].to_broadcast([1, L]), op=ALU.is_lt)
                nc.vector.tensor_scalar(lm_row[:], lm_row[:], 1.0, 30000.0,
                                        op0=ALU.subtract, op1=ALU.mult)
                eR = [[], []]   # bf16 natural [128, 300] x2 chunks per sentence
                for s in range(2):
                    for c in range(2):
                        en = gat.tile([P, EMBED], BF16, name=f"eN{s}{c}", tag=f"eN{s}{c}")
                        nc.gpsimd.indirect_dma_start(
                            out=en[:], out_offset=None, in_=emb_d[:],
                            in_offset=bass.IndirectOffsetOnAxis(ap=sT[s][c][:, b:b + 1], axis=0),
                        )
                        eR[s].append(en)
                # eT2[k]: [100, 2L] transposed embeddings, col block s*L + c*P.
                # Both c-chunk transposes land in one PSUM tile -> one copy.
                eT2 = [eTp.tile([100, L2], BF16, name=f"eT{k}", tag=f"eT{k}") for k in range(3)]
                for s in range(2):
                    for k, (k0, k1) in enumerate(EK):
                        tp = psH.tile([P, L], BF16, name="tr_ps", tag="h")
                        for c in range(2):
                            nc.tensor.transpose(tp[:100, c * P:(c + 1) * P], eR[s][c][:, k0:k1], ident_b[:])
                        nc.vector.tensor_copy(eT2[k][:, s * L:(s + 1) * L], tp[:100, :])
                # attend MLP over both sentences at once (N=512)
                ha2 = []
                for m, (m0, m1) in enumerate(H2):
                    pp = psH.tile([100, L2], F32, name="h1_ps", tag="h")
                    for k in range(3):
                        nc.tensor.matmul(pp[:], W1a_t[k][:, m0:m1], eT2[k][:],
                                         start=(k == 0), stop=(k == 2))
                    h = hp.tile([100, L2], BF16, name=f"ha{m}", tag=f"ha{m}")
                    nc.scalar.activation(h[:], pp[:], ACTF.Relu, bias=b1a_t[m][:], scale=1.0)
                    ha2.append(h)
                hT2 = []
                for m, (m0, m1) in enumerate(H2):
                    qp = psH.tile([100, L2], F32, name="h2_ps", tag="h")
                    for k2 in range(2):
                        nc.tensor.matmul(qp[:], W2a_t[k2][:, m0:m1], ha2[k2][:],
                                         start=(k2 == 0), stop=(k2 == 1))
                    h = hp.tile([100, L2], BF16, name=f"hT{m}", tag=f"hT{m}")
                    nc.scalar.activation(h[:], qp[:], ACTF.Relu, bias=b2a_t[m][:], scale=1.0)
                    hT2.append(h)

                # scores: e [i, j] and e^T [j, i]; stay in PSUM (rowmax, subtract
                # and exp all read PSUM directly - no SBUF staging copy)
                e_ps, eT_ps = [], []
                for ic in range(2):
                    ep = psS.tile([P, L], F32, name=f"e_ps{ic}", tag="score")
                    for m in range(2):
                        nc.tensor.matmul(ep[:], hT2[m][:, ic * P:(ic + 1) * P], hT2[m][:, L:L2],
                                         start=(m == 0), stop=(m == 1))
                    e_ps.append(ep)
                for jc in range(2):
                    ep = psS.tile([P, L], F32, name=f"eT_ps{jc}", tag="score")
                    for m in range(2):
                        nc.tensor.matmul(ep[:], hT2[m][:, L + jc * P:L + (jc + 1) * P], hT2[m][:, 0:L],
                                         start=(m == 0), stop=(m == 1))
                    eT_ps.append(ep)

                # exp(e) * mask1[i];  exp(eT) * mask2[j].
                # The reference's max-subtraction is skipped entirely: any shared
                # M cancels exactly between numerator and denominator, and here
                # e is bounded (~|e| < 10) so exp cannot overflow.  ACT reads the
                # score PSUM directly.
                u = [[], []]  # u[0]=uA (i-part), u[1]=uB (j-part)
                for d, (eps, lmTs) in enumerate(((e_ps, lmT[0]), (eT_ps, lmT[1]))):
                    for c in range(2):
                        uu = sm.tile([P, L], BF16, name=f"u{d}{c}", tag=f"u{d}{c}")
                        nc.scalar.activation(uu[:], eps[c][:], ACTF.Exp, bias=lmTs[c][:, b:b + 1], scale=1.0)
                        u[d].append(uu)

                # attention sums: alphas^T = e1^T-chunks @ uA, betas^T = e2-chunks @ uB.
                # Written into xT2 col block (1-d)*L so block 0 = betasT (pairs
                # with sentence 0) and block L = alphasT (pairs with sentence 1).
                # The m3=2 chunk uses the ones-augmented lhsT [128, 101]: its
                # output row 100 is the softmax denominator, reciprocal'd on ACT
                # and partition-broadcast on GpSimd into the normalizer R.
                # denominators, computed TRANSPOSED ([128, 2] instead of [1, 256])
                # so the DVE reciprocal runs on a free dim of 2, not 256.  The
                # output-column mask on alphas/betas is redundant (the compare
                # stage re-masks those columns), so R is the plain reciprocal.
                R_bc = []
                for d in range(2):
                    dpT = psA.tile([P, 2], F32, name=f"denT_ps{d}", tag="mm")
                    for jc in range(2):
                        for c in range(2):
                            nc.tensor.matmul(dpT[:, jc:jc + 1], u[d][c][:, jc * P:(jc + 1) * P],
                                             ones_col_b[:], start=(c == 0), stop=(c == 1))
                    rinvT = sm.tile([P, 2], BF16, name=f"rinvT{d}", tag=f"rinvT{d}")
                    with nc.allow_low_precision(reason="bf16 reciprocal; 0.4% rel err acceptable"):
                        nc.vector.reciprocal(rinvT[:], dpT[:])
                    rrow_ps = psA.tile([1, L], BF16, name=f"rrow_ps{d}", tag="mm")
                    with nc.allow_low_precision(reason="bf16 PE transpose of reciprocals; no accumulation"):
                        for jc in range(2):
                            nc.tensor.transpose(rrow_ps[:, jc * P:(jc + 1) * P],
                                                rinvT[:, jc:jc + 1], ident_b[:])
                    rrow = sm.tile([1, L], F32, name=f"rrow{d}", tag=f"rrow{d}")
                    nc.any.tensor_copy(rrow[:], rrow_ps[:])
                    rb = sm.tile([P, L], F32, name=f"R_bc{d}", tag=f"R_bc{d}")
                    nc.gpsimd.partition_broadcast(rb[:], rrow[:])
                    R_bc.append(rb)

                # attention sums: alphas^T = e1^T-chunks @ uA, betas^T = e2-chunks @ uB.
                # Written into xT2 col block (1-d)*L so block 0 = betasT (pairs
                # with sentence 0) and block L = alphasT (pairs with sentence 1).
                xT2 = [att.tile([100, L2], BF16, name=f"xT{m3}", tag=f"xT{m3}") for m3 in range(3)]
                for d in range(2):
                    for m3, (m0, m1) in enumerate(E3):
                        ap_ = psA.tile([100, L], F32, name="attn_ps", tag="mm")
                        for c in range(2):
                            nc.tensor.matmul(ap_[:], eR[d][c][:, m0:m1], u[d][c][:],
                                             start=(c == 0), stop=(c == 1))
                        nc.vector.tensor_tensor(xT2[m3][:, (1 - d) * L:(2 - d) * L], ap_[:],
                                                R_bc[d][:100, :], op=ALU.mult)

                # compare MLP over both sentences at once (N=512) + masked sum
                r1 = []
                for m, (m0, m1) in enumerate(H2):
                    up = psA.tile([100, L2], F32, name="c1_ps", tag="mm")
                    for k in range(3):
                        nc.tensor.matmul(up[:], W1ca_t[k][:, m0:m1], eT2[k][:],
                                         start=(k == 0), stop=False)
                    for k3 in range(3):
                        nc.tensor.matmul(up[:], W1cb_t[k3][:, m0:m1], xT2[k3][:],
                                         start=False, stop=(k3 == 2))
                    r = cmp_.tile([100, L2], BF16, name=f"r1{m}", tag=f"r1{m}")
                    nc.scalar.activation(r[:], up[:], ACTF.Relu, bias=b1c_t[m][:], scale=1.0)
                    r1.append(r)
                for m, (m0, m1) in enumerate(H2):
                    cp = psA.tile([100, L2], F32, name="c2_ps", tag="mm")
                    for k2 in range(2):
                        nc.tensor.matmul(cp[:], W2c_t[k2][:, m0:m1], r1[k2][:],
                                         start=(k2 == 0), stop=False)
                    # fold -30000 into padded columns so relu outputs exact 0
                    # there: the masked sum becomes a plain reduce.
                    nc.tensor.matmul(cp[:], ones_row_b[:, :100], lm_row[:],
                                     start=False, stop=True)
                    c2 = cmp_.tile([100, L2], F32, name=f"c2{m}", tag=f"c2{m}")
                    nc.scalar.activation(c2[:], cp[:], ACTF.Relu, bias=b2c_t[m][:], scale=1.0)
                    for s in range(2):
                        nc.vector.tensor_reduce(v_all[s][m][:, b:b + 1], c2[:, s * L:(s + 1) * L],
                                                axis=AX.X, op=ALU.add)

            # ---------------- aggregate ----------------
            vr = []
            for s in range(2):
                for m in range(2):
                    t = const.tile([100, nb], BF16, name=f"vr{s}{m}", tag=f"vr{s}{m}")
                    nc.vector.tensor_copy(t[:], v_all[s][m][:])
                    vr.append(t)
            if DBG:
                for i in range(4):
                    nc.sync.dma_start(vdbg_d[i * 100:(i + 1) * 100, :], v_all[i // 2][i % 2][:])
            g1 = []
            for m, (m0, m1) in enumerate(H2):
                gp = psA.tile([100, nb], F32, name="g_ps", tag="mm")
                for k in range(4):
                    nc.tensor.matmul(gp[:], W1g_t[k][:, m0:m1], vr[k][:],
                                     start=(k == 0), stop=(k == 3))
                g = const.tile([100, nb], BF16, name=f"g1{m}", tag=f"g1{m}")
                nc.scalar.activation(g[:], gp[:], ACTF.Relu, bias=b1g_t[m][:], scale=1.0)
                g1.append(g)
            op = psA.tile([2, nb], F32, name="o_ps", tag="mm")
            for k2 in range(2):
                nc.tensor.matmul(op[:], W2g_t[k2][:], g1[k2][:],
                                 start=(k2 == 0), stop=(k2 == 1))
            osb = const.tile([2, nb], F32, name="osb", tag="osb")
            nc.scalar.activation(osb[:], op[:], ACTF.Identity, bias=b2g_t[:], scale=1.0)
            nc.sync.dma_start(out_d[:].rearrange("b o -> o b"), osb[:])

    nc.compile()
    return nc


def _shard_inputs(inputs, nb=BC, ncores=NCORES):
    import ml_dtypes
    bf16 = ml_dtypes.bfloat16
    f = np.ascontiguousarray
    emb_b = f(inputs['emb'].astype(bf16))
    Ws = {k: f(inputs[k].astype(bf16)) for k in ('W1a', 'W2a', 'W1c', 'W2c', 'W1g', 'W2g')}
    bs = {k: f(inputs[k].reshape(-1, 1).astype(np.float32))
          for k in ('b1a', 'b2a', 'b1c', 'b2c', 'b1g', 'b2g')}
    maps = []
    for c in range(ncores):
        sl = slice(c * nb, (c + 1) * nb)
        maps.append(dict(
            emb=emb_b,
            s1=f(inputs['s1'][sl].astype(np.int32)),
            s2=f(inputs['s2'][sl].astype(np.int32)),
            len1=f(inputs['len1'][sl].reshape(nb, 1).astype(np.int32)),
            len2=f(inputs['len2'][sl].reshape(nb, 1).astype(np.int32)),
            **Ws, **bs,
        ))
    return maps


def kernel(**inputs):
    from concourse.bass_utils import run_bass_kernel_spmd
    if 'prog' not in _prog_cache:
        _prog_cache['prog'] = build_program(BC)
    nc = _prog_cache['prog']
    in_maps = _shard_inputs(inputs)
    res = run_bass_kernel_spmd(nc, in_maps, core_ids=list(range(NCORES)))
    out = np.concatenate([res.results[c]["out"] for c in range(NCORES)], axis=0)
    return out.astype(np.float32)
